# revision 3
# baseline (speedup 1.0000x reference)
"""MoE router kernel for Trainium2 (8 NeuronCores, SPMD data-parallel).

Problem: hidden_states [4, 4096, 2048] f32, W [2048, 64] f32, b [64] f32.
  logits = hidden @ W + b ; probs = sigmoid(logits)
  top-8 over experts -> (probs_topk normalized [B,S,8], indices [B,S,8] i32,
                         dense routing_map [B,S,64])

Sharding: tokens (batch*seq = 16384) split evenly across 8 cores (2048 each).
W/b replicated. No collectives.

Per-core pipeline (all fp32 on the PE for exact-enough top-k selection):
  - DMA x tiles [128, 2048] (tokens on partitions)
  - PE transpose 128x128 chunks -> PSUM -> copy to SBUF (h on partitions)
  - fp32 matmul: lhsT = W chunk [128h, 64e] stationary, rhs = xT [128h, 512t],
    accumulate 16 chunks into PSUM logits_T [64e, 512t]
  - ACT Identity(+bias b per-partition) PSUM->SBUF
  - PE transpose [64,128] -> [128t, 64e]; top-8 of logits == top-8 of probs
    (sigmoid monotonic) via DVE max / max_index (matches lax.top_k tie rules)
  - sigmoid on the 8 winners + full tile, normalize, threshold-scatter
"""

import numpy as np

import concourse.bass as bass
import concourse.mybir as mybir
import concourse.tile as tile
from concourse import bacc, bass_utils
from concourse.masks import make_identity

HIDDEN = 2048
NUM_EXPERTS = 64
TOPK = 8
N_CORES = 8
B, S = 4, 4096
TOKENS = B * S                      # 16384
TOK_PER_CORE = TOKENS // N_CORES    # 2048
P = 128                             # partitions / tile rows
SUP = 512                           # tokens per supertile (max fp32 moving dim)
N_SUP = TOK_PER_CORE // SUP         # 4
TT_PER_SUP = SUP // P               # 4
KC = HIDDEN // P                    # 16 contraction chunks

FP32 = mybir.dt.float32


def build_nc():
    nc = bacc.Bacc("TRN2", target_bir_lowering=False, debug=False,
                   num_devices=N_CORES)

    x_d = nc.dram_tensor("x", [TOK_PER_CORE, HIDDEN], FP32, kind="ExternalInput")
    w_d = nc.dram_tensor("w", [HIDDEN, NUM_EXPERTS], FP32, kind="ExternalInput")
    b_d = nc.dram_tensor("b", [NUM_EXPERTS], FP32, kind="ExternalInput")

    probs_d = nc.dram_tensor("probs", [TOK_PER_CORE, TOPK], FP32,
                             kind="ExternalOutput")
    idx_d = nc.dram_tensor("idx", [TOK_PER_CORE, TOPK], mybir.dt.int32,
                           kind="ExternalOutput")
    routing_d = nc.dram_tensor("routing", [TOK_PER_CORE, NUM_EXPERTS], FP32,
                               kind="ExternalOutput")

    NG = N_SUP * TT_PER_SUP  # 16 token groups of 128

    with tile.TileContext(nc) as tc:
        with (
            tc.tile_pool(name="const", bufs=1) as const_pool,
            tc.tile_pool(name="xin", bufs=8) as x_pool,
            tc.tile_pool(name="xt", bufs=4) as xt_pool,
            tc.tile_pool(name="z", bufs=2) as z_pool,
            tc.tile_pool(name="zt", bufs=2) as zt_pool,
            tc.tile_pool(name="small", bufs=4) as small_pool,
            tc.tile_pool(name="acc", bufs=1) as acc_pool,
            tc.tile_pool(name="psx", bufs=3, space=bass.MemorySpace.PSUM) as psx_pool,
            tc.tile_pool(name="psl", bufs=2, space=bass.MemorySpace.PSUM) as psl_pool,
            tc.tile_pool(name="psz", bufs=2, space=bass.MemorySpace.PSUM) as psz_pool,
        ):
            ident = const_pool.tile([P, P], FP32)
            make_identity(nc, ident[:])

            w_sb = const_pool.tile([P, KC * NUM_EXPERTS], FP32)
            for c in range(KC):
                nc.sync.dma_start(
                    w_sb[:, c * NUM_EXPERTS:(c + 1) * NUM_EXPERTS],
                    w_d.ap()[c * P:(c + 1) * P, :],
                )
            b_sb = const_pool.tile([NUM_EXPERTS, 1], FP32)
            nc.sync.dma_start(
                b_sb[:], b_d.ap().rearrange("(e one) -> e one", one=1)
            )

            ptop_acc = acc_pool.tile([P, NG * TOPK], FP32)
            idx_acc = acc_pool.tile([P, NG * TOPK], mybir.dt.uint32)
            rout_acc = acc_pool.tile([P, NG * NUM_EXPERTS], FP32)

            for st in range(N_SUP):
                x_tiles = []
                for tt in range(TT_PER_SUP):
                    xt_in = x_pool.tile([P, HIDDEN], FP32, tag="xin")
                    r0 = st * SUP + tt * P
                    nc.sync.dma_start(xt_in[:], x_d.ap()[r0:r0 + P, :])
                    x_tiles.append(xt_in)

                logits_ps = psl_pool.tile([NUM_EXPERTS, SUP], FP32)

                for c in range(KC):
                    xt_ps = psx_pool.tile([P, SUP], FP32)
                    for tt in range(TT_PER_SUP):
                        nc.tensor.transpose(
                            xt_ps[:, tt * P:(tt + 1) * P],
                            x_tiles[tt][:, c * P:(c + 1) * P],
                            ident[:],
                        )
                    xt_sb = xt_pool.tile([P, SUP], FP32, tag="xt")
                    if c % 2 == 0:
                        nc.scalar.copy(xt_sb[:], xt_ps[:])
                    else:
                        nc.vector.tensor_copy(xt_sb[:], xt_ps[:])
                    nc.tensor.matmul(
                        logits_ps[:],
                        w_sb[:, c * NUM_EXPERTS:(c + 1) * NUM_EXPERTS],
                        xt_sb[:],
                        start=(c == 0),
                        stop=(c == KC - 1),
                    )

                # z = logits + b, experts on partitions  [64, 512]
                z_sb = z_pool.tile([NUM_EXPERTS, SUP], FP32, tag="z")
                nc.scalar.activation(
                    z_sb[:], logits_ps[:],
                    mybir.ActivationFunctionType.Identity, bias=b_sb[:],
                )

                for tt in range(TT_PER_SUP):
                    g = st * TT_PER_SUP + tt
                    zt_ps = psz_pool.tile([P, NUM_EXPERTS], FP32)
                    nc.tensor.transpose(
                        zt_ps[:],
                        z_sb[:, tt * P:(tt + 1) * P],
                        ident[:NUM_EXPERTS, :NUM_EXPERTS],
                    )
                    zt_sb = zt_pool.tile([P, NUM_EXPERTS], FP32, tag="zt")
                    nc.vector.tensor_copy(zt_sb[:], zt_ps[:])

                    v8 = small_pool.tile([P, TOPK], FP32, tag="v8")
                    nc.vector.max(out=v8[:], in_=zt_sb[:])
                    nc.vector.max_index(
                        out=idx_acc[:, g * TOPK:(g + 1) * TOPK],
                        in_max=v8[:],
                        in_values=zt_sb[:],
                    )

                    p8 = small_pool.tile([P, TOPK], FP32, tag="p8")
                    nc.scalar.activation(
                        p8[:], v8[:], mybir.ActivationFunctionType.Sigmoid
                    )
                    s1 = small_pool.tile([P, 1], FP32, tag="s1")
                    nc.vector.reduce_sum(s1[:], p8[:], axis=mybir.AxisListType.X)
                    rec = small_pool.tile([P, 1], FP32, tag="rec")
                    nc.vector.reciprocal(rec[:], s1[:])
                    nc.vector.tensor_scalar_mul(
                        ptop_acc[:, g * TOPK:(g + 1) * TOPK], p8[:], rec[:]
                    )

                    sig_all = small_pool.tile([P, NUM_EXPERTS], FP32, tag="sig")
                    nc.scalar.activation(
                        sig_all[:], zt_sb[:], mybir.ActivationFunctionType.Sigmoid
                    )
                    selrec = small_pool.tile([P, NUM_EXPERTS], FP32, tag="sel")
                    nc.vector.tensor_scalar(
                        selrec[:],
                        zt_sb[:],
                        v8[:, TOPK - 1:TOPK],
                        rec[:],
                        op0=mybir.AluOpType.is_ge,
                        op1=mybir.AluOpType.mult,
                    )
                    nc.vector.tensor_mul(
                        rout_acc[:, g * NUM_EXPERTS:(g + 1) * NUM_EXPERTS],
                        selrec[:], sig_all[:],
                    )

            nc.sync.dma_start(
                probs_d.ap().rearrange("(g p) k -> p g k", p=P),
                ptop_acc[:].rearrange("p (g k) -> p g k", k=TOPK),
            )
            nc.sync.dma_start(
                idx_d.ap().rearrange("(g p) k -> p g k", p=P),
                idx_acc[:].bitcast(mybir.dt.int32).rearrange("p (g k) -> p g k", k=TOPK),
            )
            nc.sync.dma_start(
                routing_d.ap().rearrange("(g p) e -> p g e", p=P),
                rout_acc[:].rearrange("p (g e) -> p g e", e=NUM_EXPERTS),
            )

    nc.compile()
    return nc


_NC_CACHE = None


def _get_nc():
    global _NC_CACHE
    if _NC_CACHE is None:
        _NC_CACHE = build_nc()
    return _NC_CACHE


def run_sharded(hidden_states, W, b, trace=False):
    nc = _get_nc()
    hs = np.ascontiguousarray(np.asarray(hidden_states, dtype=np.float32))
    W = np.ascontiguousarray(np.asarray(W, dtype=np.float32))
    b = np.ascontiguousarray(np.asarray(b, dtype=np.float32))
    flat = hs.reshape(TOKENS, HIDDEN)
    in_maps = [
        {
            "x": flat[c * TOK_PER_CORE:(c + 1) * TOK_PER_CORE],
            "w": W,
            "b": b,
        }
        for c in range(N_CORES)
    ]
    out = bass_utils.run_bass_kernel_spmd(
        nc, in_maps, core_ids=list(range(N_CORES)), trace=trace
    )
    return out


def kernel(hidden_states, W, b):
    out = run_sharded(hidden_states, W, b)
    res = out.results
    probs = np.concatenate([r["probs"] for r in res], axis=0)
    idx = np.concatenate([r["idx"] for r in res], axis=0)
    routing = np.concatenate([r["routing"] for r in res], axis=0)
    probs_topk = probs.reshape(B, S, TOPK).astype(np.float32)
    indices_topk = idx.reshape(B, S, TOPK).astype(np.int32)
    routing_map = routing.reshape(B, S, NUM_EXPERTS).astype(np.float32)
    return probs_topk, indices_topk, routing_map


# revision 6
# speedup vs baseline: 91.0834x; 91.0834x over previous
"""MoE router kernel for Trainium2 (8 NeuronCores, SPMD data-parallel).

Problem: hidden_states [4, 4096, 2048] f32, W [2048, 64] f32, b [64] f32.
  logits = hidden @ W + b ; probs = sigmoid(logits)
  top-8 over experts -> (probs_topk normalized [B,S,8], indices [B,S,8] i32,
                         dense routing_map [B,S,64])

Sharding: tokens (batch*seq = 16384) split evenly across 8 cores (2048 each).
W/b replicated. No collectives.

Per-core pipeline (all fp32 on the PE for exact-enough top-k selection):
  - DMA x tiles [128, 2048] (tokens on partitions)
  - PE transpose 128x128 chunks -> PSUM -> copy to SBUF (h on partitions)
  - fp32 matmul: lhsT = W chunk [128h, 64e] stationary, rhs = xT [128h, 512t],
    accumulate 16 chunks into PSUM logits_T [64e, 512t]
  - ACT Identity(+bias b per-partition) PSUM->SBUF
  - PE transpose [64,128] -> [128t, 64e]; top-8 of logits == top-8 of probs
    (sigmoid monotonic) via DVE max / max_index (matches lax.top_k tie rules)
  - sigmoid on the 8 winners + full tile, normalize, threshold-scatter
"""

import numpy as np

import concourse.bass as bass
import concourse.mybir as mybir
import concourse.tile as tile
from concourse import bacc, bass_utils
from concourse.masks import make_identity

HIDDEN = 2048
NUM_EXPERTS = 64
TOPK = 8
N_CORES = 8
B, S = 4, 4096
TOKENS = B * S                      # 16384
TOK_PER_CORE = TOKENS // N_CORES    # 2048
P = 128                             # partitions / tile rows
SUP = 512                           # tokens per supertile (max fp32 moving dim)
N_SUP = TOK_PER_CORE // SUP         # 4
TT_PER_SUP = SUP // P               # 4
KC = HIDDEN // P                    # 16 contraction chunks

FP32 = mybir.dt.float32


def build_nc(loop_iters=None):
    """Build the per-core program. loop_iters wraps the whole body in an
    on-device For_i repeat loop (used only for wall-clock timing)."""
    nc = bacc.Bacc("TRN2", target_bir_lowering=False, debug=False,
                   num_devices=N_CORES)

    x_d = nc.dram_tensor("x", [TOK_PER_CORE, HIDDEN], FP32, kind="ExternalInput")
    w_d = nc.dram_tensor("w", [HIDDEN, NUM_EXPERTS], FP32, kind="ExternalInput")
    b_d = nc.dram_tensor("b", [NUM_EXPERTS], FP32, kind="ExternalInput")

    probs_d = nc.dram_tensor("probs", [TOK_PER_CORE, TOPK], FP32,
                             kind="ExternalOutput")
    idx_d = nc.dram_tensor("idx", [TOK_PER_CORE, TOPK], mybir.dt.int32,
                           kind="ExternalOutput")
    routing_d = nc.dram_tensor("routing", [TOK_PER_CORE, NUM_EXPERTS], FP32,
                               kind="ExternalOutput")

    NG = N_SUP * TT_PER_SUP  # 16 token groups of 128

    with tile.TileContext(nc) as tc:
        with (
            tc.tile_pool(name="const", bufs=1) as const_pool,
            tc.tile_pool(name="xin", bufs=8) as x_pool,
            tc.tile_pool(name="xt", bufs=4) as xt_pool,
            tc.tile_pool(name="z", bufs=2) as z_pool,
            tc.tile_pool(name="zt", bufs=2) as zt_pool,
            tc.tile_pool(name="small", bufs=4) as small_pool,
            tc.tile_pool(name="acc", bufs=1) as acc_pool,
            tc.tile_pool(name="psx", bufs=3, space=bass.MemorySpace.PSUM) as psx_pool,
            tc.tile_pool(name="psl", bufs=2, space=bass.MemorySpace.PSUM) as psl_pool,
            tc.tile_pool(name="psz", bufs=2, space=bass.MemorySpace.PSUM) as psz_pool,
        ):
            ident = const_pool.tile([P, P], FP32)
            make_identity(nc, ident[:])

            w_sb = const_pool.tile([P, KC * NUM_EXPERTS], FP32)
            for c in range(KC):
                nc.sync.dma_start(
                    w_sb[:, c * NUM_EXPERTS:(c + 1) * NUM_EXPERTS],
                    w_d.ap()[c * P:(c + 1) * P, :],
                )
            b_sb = const_pool.tile([NUM_EXPERTS, 1], FP32)
            nc.sync.dma_start(
                b_sb[:], b_d.ap().rearrange("(e one) -> e one", one=1)
            )

            ptop_acc = acc_pool.tile([P, NG * TOPK], FP32)
            idx_acc = acc_pool.tile([P, NG * TOPK], mybir.dt.uint32)
            rout_acc = acc_pool.tile([P, NG * NUM_EXPERTS], FP32)

            from contextlib import nullcontext
            loop_cm = (
                tc.For_i(0, loop_iters, 1,
                         hint_engines=(mybir.EngineType.PE,
                                       mybir.EngineType.Activation,
                                       mybir.EngineType.DVE,
                                       mybir.EngineType.SP))
                if loop_iters else nullcontext()
            )
            with loop_cm:
                _emit_body(nc, tc, x_d, probs_d, idx_d, routing_d, ident, w_sb,
                           b_sb, ptop_acc, idx_acc, rout_acc,
                           x_pool, xt_pool, z_pool, zt_pool, small_pool,
                           psx_pool, psl_pool, psz_pool)

    nc.compile()
    return nc


def _emit_body(nc, tc, x_d, probs_d, idx_d, routing_d, ident, w_sb, b_sb,
               ptop_acc, idx_acc, rout_acc, x_pool, xt_pool, z_pool, zt_pool,
               small_pool, psx_pool, psl_pool, psz_pool):
    NG = N_SUP * TT_PER_SUP

    if True:
            for st in range(N_SUP):
                x_tiles = []
                for tt in range(TT_PER_SUP):
                    xt_in = x_pool.tile([P, HIDDEN], FP32, tag="xin")
                    r0 = st * SUP + tt * P
                    nc.sync.dma_start(xt_in[:], x_d.ap()[r0:r0 + P, :])
                    x_tiles.append(xt_in)

                logits_ps = psl_pool.tile([NUM_EXPERTS, SUP], FP32)

                for c in range(KC):
                    xt_ps = psx_pool.tile([P, SUP], FP32)
                    for tt in range(TT_PER_SUP):
                        nc.tensor.transpose(
                            xt_ps[:, tt * P:(tt + 1) * P],
                            x_tiles[tt][:, c * P:(c + 1) * P],
                            ident[:],
                        )
                    xt_sb = xt_pool.tile([P, SUP], FP32, tag="xt")
                    if c % 2 == 0:
                        nc.scalar.copy(xt_sb[:], xt_ps[:])
                    else:
                        nc.vector.tensor_copy(xt_sb[:], xt_ps[:])
                    nc.tensor.matmul(
                        logits_ps[:],
                        w_sb[:, c * NUM_EXPERTS:(c + 1) * NUM_EXPERTS],
                        xt_sb[:],
                        start=(c == 0),
                        stop=(c == KC - 1),
                    )

                # z = logits + b, experts on partitions  [64, 512]
                z_sb = z_pool.tile([NUM_EXPERTS, SUP], FP32, tag="z")
                nc.scalar.activation(
                    z_sb[:], logits_ps[:],
                    mybir.ActivationFunctionType.Identity, bias=b_sb[:],
                )

                for tt in range(TT_PER_SUP):
                    g = st * TT_PER_SUP + tt
                    zt_ps = psz_pool.tile([P, NUM_EXPERTS], FP32)
                    nc.tensor.transpose(
                        zt_ps[:],
                        z_sb[:, tt * P:(tt + 1) * P],
                        ident[:NUM_EXPERTS, :NUM_EXPERTS],
                    )
                    zt_sb = zt_pool.tile([P, NUM_EXPERTS], FP32, tag="zt")
                    nc.vector.tensor_copy(zt_sb[:], zt_ps[:])

                    v8 = small_pool.tile([P, TOPK], FP32, tag="v8")
                    nc.vector.max(out=v8[:], in_=zt_sb[:])
                    nc.vector.max_index(
                        out=idx_acc[:, g * TOPK:(g + 1) * TOPK],
                        in_max=v8[:],
                        in_values=zt_sb[:],
                    )

                    p8 = small_pool.tile([P, TOPK], FP32, tag="p8")
                    nc.scalar.activation(
                        p8[:], v8[:], mybir.ActivationFunctionType.Sigmoid
                    )
                    s1 = small_pool.tile([P, 1], FP32, tag="s1")
                    nc.vector.reduce_sum(s1[:], p8[:], axis=mybir.AxisListType.X)
                    rec = small_pool.tile([P, 1], FP32, tag="rec")
                    nc.vector.reciprocal(rec[:], s1[:])
                    nc.vector.tensor_scalar_mul(
                        ptop_acc[:, g * TOPK:(g + 1) * TOPK], p8[:], rec[:]
                    )

                    sig_all = small_pool.tile([P, NUM_EXPERTS], FP32, tag="sig")
                    nc.scalar.activation(
                        sig_all[:], zt_sb[:], mybir.ActivationFunctionType.Sigmoid
                    )
                    selrec = small_pool.tile([P, NUM_EXPERTS], FP32, tag="sel")
                    nc.vector.tensor_scalar(
                        selrec[:],
                        zt_sb[:],
                        v8[:, TOPK - 1:TOPK],
                        rec[:],
                        op0=mybir.AluOpType.is_ge,
                        op1=mybir.AluOpType.mult,
                    )
                    nc.vector.tensor_mul(
                        rout_acc[:, g * NUM_EXPERTS:(g + 1) * NUM_EXPERTS],
                        selrec[:], sig_all[:],
                    )

            nc.sync.dma_start(
                probs_d.ap().rearrange("(g p) k -> p g k", p=P),
                ptop_acc[:].rearrange("p (g k) -> p g k", k=TOPK),
            )
            nc.sync.dma_start(
                idx_d.ap().rearrange("(g p) k -> p g k", p=P),
                idx_acc[:].bitcast(mybir.dt.int32).rearrange("p (g k) -> p g k", k=TOPK),
            )
            nc.sync.dma_start(
                routing_d.ap().rearrange("(g p) e -> p g e", p=P),
                rout_acc[:].rearrange("p (g e) -> p g e", e=NUM_EXPERTS),
            )


_NC_CACHE = None


def _get_nc():
    global _NC_CACHE
    if _NC_CACHE is None:
        _NC_CACHE = build_nc()
    return _NC_CACHE


def run_sharded(hidden_states, W, b, trace=False):
    nc = _get_nc()
    hs = np.ascontiguousarray(np.asarray(hidden_states, dtype=np.float32))
    W = np.ascontiguousarray(np.asarray(W, dtype=np.float32))
    b = np.ascontiguousarray(np.asarray(b, dtype=np.float32))
    flat = hs.reshape(TOKENS, HIDDEN)
    in_maps = [
        {
            "x": flat[c * TOK_PER_CORE:(c + 1) * TOK_PER_CORE],
            "w": W,
            "b": b,
        }
        for c in range(N_CORES)
    ]
    out = bass_utils.run_bass_kernel_spmd(
        nc, in_maps, core_ids=list(range(N_CORES)), trace=trace
    )
    return out


def kernel(hidden_states, W, b):
    out = run_sharded(hidden_states, W, b)
    res = out.results
    probs = np.concatenate([r["probs"] for r in res], axis=0)
    idx = np.concatenate([r["idx"] for r in res], axis=0)
    routing = np.concatenate([r["routing"] for r in res], axis=0)
    probs_topk = probs.reshape(B, S, TOPK).astype(np.float32)
    indices_topk = idx.reshape(B, S, TOPK).astype(np.int32)
    routing_map = routing.reshape(B, S, NUM_EXPERTS).astype(np.float32)
    return probs_topk, indices_topk, routing_map


# revision 19
# speedup vs baseline: 92.0832x; 1.0110x over previous
"""MoE router kernel for Trainium2 (8 NeuronCores, SPMD data-parallel).

Problem: hidden_states [4, 4096, 2048] f32, W [2048, 64] f32, b [64] f32.
  logits = hidden @ W + b ; probs = sigmoid(logits)
  top-8 over experts -> (probs_topk normalized [B,S,8], indices [B,S,8] i32,
                         dense routing_map [B,S,64])

Sharding: tokens (batch*seq = 16384) split evenly across 8 cores (2048 each).
W/b replicated. No collectives.

Per-core pipeline (all fp32 on the PE for exact-enough top-k selection):
  - DMA x tiles [128, 2048] (tokens on partitions)
  - PE transpose 128x128 chunks -> PSUM -> copy to SBUF (h on partitions)
  - fp32 matmul: lhsT = W chunk [128h, 64e] stationary, rhs = xT [128h, 512t],
    accumulate 16 chunks into PSUM logits_T [64e, 512t]
  - ACT Identity(+bias b per-partition) PSUM->SBUF
  - PE transpose [64,128] -> [128t, 64e]; top-8 of logits == top-8 of probs
    (sigmoid monotonic) via DVE max / max_index (matches lax.top_k tie rules)
  - sigmoid on the 8 winners + full tile, normalize, threshold-scatter
"""

import numpy as np

import concourse.bass as bass
import concourse.mybir as mybir
import concourse.tile as tile
from concourse import bacc, bass_utils
from concourse.masks import make_identity

HIDDEN = 2048
NUM_EXPERTS = 64
TOPK = 8
N_CORES = 8
B, S = 4, 4096
TOKENS = B * S                      # 16384
TOK_PER_CORE = TOKENS // N_CORES    # 2048
P = 128                             # partitions / tile rows
SUP = 512                           # tokens per supertile (max fp32 moving dim)
N_SUP = TOK_PER_CORE // SUP         # 4
TT_PER_SUP = SUP // P               # 4
KC = HIDDEN // P                    # 16 contraction chunks

FP32 = mybir.dt.float32


def build_nc(loop_iters=None):
    """Build the per-core program. loop_iters wraps the whole body in an
    on-device For_i repeat loop (used only for wall-clock timing)."""
    nc = bacc.Bacc("TRN2", target_bir_lowering=False, debug=False,
                   num_devices=N_CORES)

    x_d = nc.dram_tensor("x", [TOK_PER_CORE, HIDDEN], FP32, kind="ExternalInput")
    w_d = nc.dram_tensor("w", [HIDDEN, NUM_EXPERTS], FP32, kind="ExternalInput")
    b_d = nc.dram_tensor("b", [NUM_EXPERTS], FP32, kind="ExternalInput")

    probs_d = nc.dram_tensor("probs", [TOK_PER_CORE, TOPK], FP32,
                             kind="ExternalOutput")
    idx_d = nc.dram_tensor("idx", [TOK_PER_CORE, TOPK], mybir.dt.int32,
                           kind="ExternalOutput")
    routing_d = nc.dram_tensor("routing", [TOK_PER_CORE, NUM_EXPERTS], FP32,
                               kind="ExternalOutput")

    NG = N_SUP * TT_PER_SUP  # 16 token groups of 128

    with tile.TileContext(nc) as tc:
        with (
            tc.tile_pool(name="const", bufs=1) as const_pool,
            tc.tile_pool(name="xin", bufs=8) as x_pool,
            tc.tile_pool(name="xt", bufs=4) as xt_pool,
            tc.tile_pool(name="z", bufs=2) as z_pool,
            tc.tile_pool(name="zt", bufs=2) as zt_pool,
            tc.tile_pool(name="small", bufs=4) as small_pool,
            tc.tile_pool(name="acc", bufs=1) as acc_pool,
            tc.tile_pool(name="psx", bufs=3, space=bass.MemorySpace.PSUM) as psx_pool,
            tc.tile_pool(name="psl", bufs=2, space=bass.MemorySpace.PSUM) as psl_pool,
            tc.tile_pool(name="psz", bufs=2, space=bass.MemorySpace.PSUM) as psz_pool,
        ):
            ident = const_pool.tile([P, P], FP32)
            make_identity(nc, ident[:])

            w_sb = const_pool.tile([P, KC * NUM_EXPERTS], FP32)
            # bias for the split accumulator [L0; L1]: add b to L0's half only
            b_sb = const_pool.tile([P, 1], FP32)
            nc.vector.memset(b_sb[NUM_EXPERTS:P, :], 0.0)

            ptop_acc = acc_pool.tile([P, NG * TOPK], FP32)
            idx_acc = acc_pool.tile([P, NG * TOPK], mybir.dt.uint32)
            rout_acc = acc_pool.tile([P, NG * NUM_EXPERTS], FP32)

            from contextlib import nullcontext
            loop_cm = (
                tc.For_i(0, loop_iters, 1,
                         hint_engines=(mybir.EngineType.PE,
                                       mybir.EngineType.Activation,
                                       mybir.EngineType.DVE,
                                       mybir.EngineType.SP))
                if loop_iters else nullcontext()
            )
            with loop_cm:
                _emit_body(nc, tc, x_d, w_d, b_d, probs_d, idx_d, routing_d,
                           ident, w_sb, b_sb, ptop_acc, idx_acc, rout_acc,
                           x_pool, xt_pool, z_pool, zt_pool, small_pool,
                           psx_pool, psl_pool, psz_pool)

    nc.compile()
    return nc


def _emit_body(nc, tc, x_d, w_d, b_d, probs_d, idx_d, routing_d, ident, w_sb,
               b_sb, ptop_acc, idx_acc, rout_acc, x_pool, xt_pool, z_pool,
               zt_pool, small_pool, psx_pool, psl_pool, psz_pool):
    NG = N_SUP * TT_PER_SUP

    def emit_topk(st, z_sb):
        """Top-8 + normalize + scatter for one supertile's z [64, 512]."""
        for tt in range(TT_PER_SUP):
            g = st * TT_PER_SUP + tt
            zt_ps = psz_pool.tile([P, P], FP32, tag="zps")
            nc.tensor.transpose(
                zt_ps[:],
                z_sb[:, tt * P:(tt + 1) * P],
                ident[:],
            )
            zt2_sb = zt_pool.tile([P, P], FP32, tag="zt2")
            nc.vector.tensor_copy(zt2_sb[:], zt_ps[:])
            # combine the split-accumulator halves: z = (L0+b)^T + L1^T
            zt_sb = zt_pool.tile([P, NUM_EXPERTS], FP32, tag="zt")
            nc.vector.tensor_add(
                zt_sb[:], zt2_sb[:, 0:NUM_EXPERTS], zt2_sb[:, NUM_EXPERTS:P]
            )

            v8 = small_pool.tile([P, TOPK], FP32, tag="v8")
            nc.vector.max(out=v8[:], in_=zt_sb[:])
            nc.vector.max_index(
                out=idx_acc[:, g * TOPK:(g + 1) * TOPK],
                in_max=v8[:],
                in_values=zt_sb[:],
            )

            p8 = small_pool.tile([P, TOPK], FP32, tag="p8")
            nc.scalar.activation(
                p8[:], v8[:], mybir.ActivationFunctionType.Sigmoid
            )
            s1 = small_pool.tile([P, 1], FP32, tag="s1")
            nc.vector.reduce_sum(s1[:], p8[:], axis=mybir.AxisListType.X)
            rec = small_pool.tile([P, 1], FP32, tag="rec")
            nc.vector.reciprocal(rec[:], s1[:])
            nc.vector.tensor_scalar_mul(
                ptop_acc[:, g * TOPK:(g + 1) * TOPK], p8[:], rec[:]
            )

            sig_all = small_pool.tile([P, NUM_EXPERTS], FP32, tag="sig")
            nc.scalar.activation(
                sig_all[:], zt_sb[:], mybir.ActivationFunctionType.Sigmoid
            )
            selrec = small_pool.tile([P, NUM_EXPERTS], FP32, tag="sel")
            nc.vector.tensor_scalar(
                selrec[:],
                zt_sb[:],
                v8[:, TOPK - 1:TOPK],
                rec[:],
                op0=mybir.AluOpType.is_ge,
                op1=mybir.AluOpType.mult,
            )
            nc.vector.tensor_mul(
                rout_acc[:, g * NUM_EXPERTS:(g + 1) * NUM_EXPERTS],
                selrec[:], sig_all[:],
            )

        # flush this supertile's outputs (keeps the kernel tail short)
        g0, g1 = st * TT_PER_SUP, (st + 1) * TT_PER_SUP
        nc.sync.dma_start(
            probs_d.ap().rearrange("(g p) k -> p g k", p=P)[:, g0:g1, :],
            ptop_acc[:].rearrange("p (g k) -> p g k", k=TOPK)[:, g0:g1, :],
        )
        nc.sync.dma_start(
            idx_d.ap().rearrange("(g p) k -> p g k", p=P)[:, g0:g1, :],
            idx_acc[:].bitcast(mybir.dt.int32)
            .rearrange("p (g k) -> p g k", k=TOPK)[:, g0:g1, :],
        )
        nc.sync.dma_start(
            routing_d.ap().rearrange("(g p) e -> p g e", p=P)[:, g0:g1, :],
            rout_acc[:].rearrange("p (g e) -> p g e", e=NUM_EXPERTS)[:, g0:g1, :],
        )

    pending_topk = None  # (st, z_sb) of the previous supertile

    for st in range(N_SUP):
        x_tiles = []
        if st == 0:
            # first supertile: load in column pieces so chunk-0 transposes can
            # start after ~1/4 of the bytes; interleave W/b right after the
            # first piece (the first matmul needs W only after 4 transposes)
            for tt in range(TT_PER_SUP):
                xt_in = x_pool.tile([P, HIDDEN], FP32, tag="xin")
                x_tiles.append(xt_in)
            NPIECE = 4
            pw = HIDDEN // NPIECE
            for tt in range(TT_PER_SUP):
                r0 = st * SUP + tt * P
                nc.sync.dma_start(
                    x_tiles[tt][:, 0:pw], x_d.ap()[r0:r0 + P, 0:pw]
                )
            nc.scalar.dma_start(
                w_sb[:].rearrange("p (c e) -> p c e", e=NUM_EXPERTS),
                w_d.ap().rearrange("(c p) e -> p c e", p=P),
            )
            nc.scalar.dma_start(
                b_sb[0:NUM_EXPERTS, :],
                b_d.ap().rearrange("(e one) -> e one", one=1),
            )
            for piece in range(1, NPIECE):
                for tt in range(TT_PER_SUP):
                    r0 = st * SUP + tt * P
                    nc.sync.dma_start(
                        x_tiles[tt][:, piece * pw:(piece + 1) * pw],
                        x_d.ap()[r0:r0 + P, piece * pw:(piece + 1) * pw],
                    )
        else:
            # later supertiles prefetch on the ACT HWDGE ring so the sync
            # ring's sequencer stays free for output flushes
            for tt in range(TT_PER_SUP):
                xt_in = x_pool.tile([P, HIDDEN], FP32, tag="xin")
                r0 = st * SUP + tt * P
                nc.scalar.dma_start(xt_in[:], x_d.ap()[r0:r0 + P, :])
                x_tiles.append(xt_in)

        # split accumulator: even chunks -> rows 0:64 (col group 0), odd
        # chunks -> rows 64:128 (col group 1); the two col-tiled matmuls run
        # concurrently in the PE array (~1.9x measured)
        logits_ps = psl_pool.tile([P, SUP], FP32)

        def emit_mm(c):
            half = c % 2
            nc.tensor.matmul(
                logits_ps[half * NUM_EXPERTS:(half + 1) * NUM_EXPERTS, :],
                w_sb[:, c * NUM_EXPERTS:(c + 1) * NUM_EXPERTS],
                xt_done[c][:],
                start=(c < 2),
                stop=(c >= KC - 2),
                tile_position=(0, half * NUM_EXPERTS),
            )

        # software pipeline: matmuls for chunk c-1 are emitted after the
        # transposes of chunk c, so the PE never waits on the PSUM->SBUF copy
        xt_done = {}
        for c in range(KC):
            xt_ps = psx_pool.tile([P, SUP], FP32)
            for tt in range(TT_PER_SUP):
                nc.tensor.transpose(
                    xt_ps[:, tt * P:(tt + 1) * P],
                    x_tiles[tt][:, c * P:(c + 1) * P],
                    ident[:],
                )
            if c >= 1:
                emit_mm(c - 1)
            xt_sb = xt_pool.tile([P, SUP], FP32, tag="xt")
            if c % 2 == 0:
                nc.scalar.copy(xt_sb[:], xt_ps[:])
            else:
                nc.vector.tensor_copy(xt_sb[:], xt_ps[:])
            xt_done[c] = xt_sb

            if c == 2 and pending_topk is not None:
                emit_topk(*pending_topk)
                pending_topk = None

        emit_mm(KC - 1)

        # zz = [L0 + b ; L1], both halves on partitions  [128, 512]
        z_sb = z_pool.tile([P, SUP], FP32, tag="z")
        nc.scalar.activation(
            z_sb[:], logits_ps[:],
            mybir.ActivationFunctionType.Identity, bias=b_sb[:],
        )
        pending_topk = (st, z_sb)

    emit_topk(*pending_topk)


_NC_CACHE = None


def _get_nc():
    global _NC_CACHE
    if _NC_CACHE is None:
        _NC_CACHE = build_nc()
    return _NC_CACHE


def run_sharded(hidden_states, W, b, trace=False):
    nc = _get_nc()
    hs = np.ascontiguousarray(np.asarray(hidden_states, dtype=np.float32))
    W = np.ascontiguousarray(np.asarray(W, dtype=np.float32))
    b = np.ascontiguousarray(np.asarray(b, dtype=np.float32))
    flat = hs.reshape(TOKENS, HIDDEN)
    in_maps = [
        {
            "x": flat[c * TOK_PER_CORE:(c + 1) * TOK_PER_CORE],
            "w": W,
            "b": b,
        }
        for c in range(N_CORES)
    ]
    out = bass_utils.run_bass_kernel_spmd(
        nc, in_maps, core_ids=list(range(N_CORES)), trace=trace
    )
    return out


def kernel(hidden_states, W, b):
    out = run_sharded(hidden_states, W, b)
    res = out.results
    probs = np.concatenate([r["probs"] for r in res], axis=0)
    idx = np.concatenate([r["idx"] for r in res], axis=0)
    routing = np.concatenate([r["routing"] for r in res], axis=0)
    probs_topk = probs.reshape(B, S, TOPK).astype(np.float32)
    indices_topk = idx.reshape(B, S, TOPK).astype(np.int32)
    routing_map = routing.reshape(B, S, NUM_EXPERTS).astype(np.float32)
    return probs_topk, indices_topk, routing_map


# revision 21
# speedup vs baseline: 100.2635x; 1.0888x over previous
"""MoE router kernel for Trainium2 (8 NeuronCores, SPMD data-parallel).

Problem: hidden_states [4, 4096, 2048] f32, W [2048, 64] f32, b [64] f32.
  logits = hidden @ W + b ; probs = sigmoid(logits)
  top-8 over experts -> (probs_topk normalized [B,S,8], indices [B,S,8] i32,
                         dense routing_map [B,S,64])

Sharding: tokens (batch*seq = 16384) split evenly across 8 cores (2048 each).
W/b replicated. No collectives.

Per-core pipeline (all fp32 on the PE for exact-enough top-k selection):
  - DMA x tiles [128, 2048] (tokens on partitions)
  - PE transpose 128x128 chunks -> PSUM -> copy to SBUF (h on partitions)
  - fp32 matmul: lhsT = W chunk [128h, 64e] stationary, rhs = xT [128h, 512t],
    accumulate 16 chunks into PSUM logits_T [64e, 512t]
  - ACT Identity(+bias b per-partition) PSUM->SBUF
  - PE transpose [64,128] -> [128t, 64e]; top-8 of logits == top-8 of probs
    (sigmoid monotonic) via DVE max / max_index (matches lax.top_k tie rules)
  - sigmoid on the 8 winners + full tile, normalize, threshold-scatter
"""

import numpy as np

import concourse.bass as bass
import concourse.mybir as mybir
import concourse.tile as tile
from concourse import bacc, bass_utils
from concourse.masks import make_identity

HIDDEN = 2048
NUM_EXPERTS = 64
TOPK = 8
N_CORES = 8
B, S = 4, 4096
TOKENS = B * S                      # 16384
TOK_PER_CORE = TOKENS // N_CORES    # 2048
P = 128                             # partitions / tile rows
SUP = 512                           # tokens per supertile (max fp32 moving dim)
N_SUP = TOK_PER_CORE // SUP         # 4
TT_PER_SUP = SUP // P               # 4
KC = HIDDEN // P                    # 16 contraction chunks

FP32 = mybir.dt.float32


def build_nc(loop_iters=None):
    """Build the per-core program. loop_iters wraps the whole body in an
    on-device For_i repeat loop (used only for wall-clock timing)."""
    nc = bacc.Bacc("TRN2", target_bir_lowering=False, debug=False,
                   num_devices=N_CORES)

    x_d = nc.dram_tensor("x", [TOK_PER_CORE, HIDDEN], FP32, kind="ExternalInput")
    w_d = nc.dram_tensor("w", [HIDDEN, NUM_EXPERTS], FP32, kind="ExternalInput")
    b_d = nc.dram_tensor("b", [NUM_EXPERTS], FP32, kind="ExternalInput")

    probs_d = nc.dram_tensor("probs", [TOK_PER_CORE, TOPK], FP32,
                             kind="ExternalOutput")
    idx_d = nc.dram_tensor("idx", [TOK_PER_CORE, TOPK], mybir.dt.int32,
                           kind="ExternalOutput")
    routing_d = nc.dram_tensor("routing", [TOK_PER_CORE, NUM_EXPERTS], FP32,
                               kind="ExternalOutput")

    NG = N_SUP * TT_PER_SUP  # 16 token groups of 128

    with tile.TileContext(nc) as tc:
        with (
            tc.tile_pool(name="const", bufs=1) as const_pool,
            tc.tile_pool(name="xin", bufs=8) as x_pool,
            tc.tile_pool(name="xt", bufs=6) as xt_pool,
            tc.tile_pool(name="z", bufs=2) as z_pool,
            tc.tile_pool(name="zt", bufs=2) as zt_pool,
            tc.tile_pool(name="small", bufs=4) as small_pool,
            tc.tile_pool(name="acc", bufs=1) as acc_pool,
            tc.tile_pool(name="psx", bufs=4, space=bass.MemorySpace.PSUM) as psx_pool,
            tc.tile_pool(name="psl", bufs=2, space=bass.MemorySpace.PSUM) as psl_pool,
            tc.tile_pool(name="psz", bufs=2, space=bass.MemorySpace.PSUM) as psz_pool,
        ):
            ident = const_pool.tile([P, P], FP32)
            make_identity(nc, ident[:])

            w_sb = const_pool.tile([P, KC * NUM_EXPERTS], FP32)
            # bias for the split accumulator [L0; L1]: add b to L0's half only
            b_sb = const_pool.tile([P, 1], FP32)
            nc.vector.memset(b_sb[NUM_EXPERTS:P, :], 0.0)

            ptop_acc = acc_pool.tile([P, NG * TOPK], FP32)
            idx_acc = acc_pool.tile([P, NG * TOPK], mybir.dt.uint32)
            rout_acc = acc_pool.tile([P, NG * NUM_EXPERTS], FP32)

            from contextlib import nullcontext
            loop_cm = (
                tc.For_i(0, loop_iters, 1,
                         hint_engines=(mybir.EngineType.PE,
                                       mybir.EngineType.Activation,
                                       mybir.EngineType.DVE,
                                       mybir.EngineType.SP))
                if loop_iters else nullcontext()
            )
            with loop_cm:
                _emit_body(nc, tc, x_d, w_d, b_d, probs_d, idx_d, routing_d,
                           ident, w_sb, b_sb, ptop_acc, idx_acc, rout_acc,
                           x_pool, xt_pool, z_pool, zt_pool, small_pool,
                           psx_pool, psl_pool, psz_pool)

    nc.compile()
    return nc


def _emit_body(nc, tc, x_d, w_d, b_d, probs_d, idx_d, routing_d, ident, w_sb,
               b_sb, ptop_acc, idx_acc, rout_acc, x_pool, xt_pool, z_pool,
               zt_pool, small_pool, psx_pool, psl_pool, psz_pool):
    NG = N_SUP * TT_PER_SUP

    def emit_topk(st, z_sb):
        """Top-8 + normalize + scatter for one supertile's z [64, 512]."""
        for tt in range(TT_PER_SUP):
            g = st * TT_PER_SUP + tt
            zt_ps = psz_pool.tile([P, P], FP32, tag="zps")
            nc.tensor.transpose(
                zt_ps[:],
                z_sb[:, tt * P:(tt + 1) * P],
                ident[:],
            )
            zt2_sb = zt_pool.tile([P, P], FP32, tag="zt2")
            nc.vector.tensor_copy(zt2_sb[:], zt_ps[:])
            # combine the split-accumulator halves: z = (L0+b)^T + L1^T
            zt_sb = zt_pool.tile([P, NUM_EXPERTS], FP32, tag="zt")
            nc.vector.tensor_add(
                zt_sb[:], zt2_sb[:, 0:NUM_EXPERTS], zt2_sb[:, NUM_EXPERTS:P]
            )

            v8 = small_pool.tile([P, TOPK], FP32, tag="v8")
            nc.vector.max(out=v8[:], in_=zt_sb[:])
            nc.vector.max_index(
                out=idx_acc[:, g * TOPK:(g + 1) * TOPK],
                in_max=v8[:],
                in_values=zt_sb[:],
            )

            p8 = small_pool.tile([P, TOPK], FP32, tag="p8")
            nc.scalar.activation(
                p8[:], v8[:], mybir.ActivationFunctionType.Sigmoid
            )
            s1 = small_pool.tile([P, 1], FP32, tag="s1")
            nc.vector.reduce_sum(s1[:], p8[:], axis=mybir.AxisListType.X)
            rec = small_pool.tile([P, 1], FP32, tag="rec")
            nc.vector.reciprocal(rec[:], s1[:])
            nc.vector.tensor_scalar_mul(
                ptop_acc[:, g * TOPK:(g + 1) * TOPK], p8[:], rec[:]
            )

            sig_all = small_pool.tile([P, NUM_EXPERTS], FP32, tag="sig")
            nc.scalar.activation(
                sig_all[:], zt_sb[:], mybir.ActivationFunctionType.Sigmoid
            )
            selrec = small_pool.tile([P, NUM_EXPERTS], FP32, tag="sel")
            nc.vector.tensor_scalar(
                selrec[:],
                zt_sb[:],
                v8[:, TOPK - 1:TOPK],
                rec[:],
                op0=mybir.AluOpType.is_ge,
                op1=mybir.AluOpType.mult,
            )
            nc.vector.tensor_mul(
                rout_acc[:, g * NUM_EXPERTS:(g + 1) * NUM_EXPERTS],
                selrec[:], sig_all[:],
            )

        # flush this supertile's outputs (keeps the kernel tail short)
        g0, g1 = st * TT_PER_SUP, (st + 1) * TT_PER_SUP
        nc.sync.dma_start(
            probs_d.ap().rearrange("(g p) k -> p g k", p=P)[:, g0:g1, :],
            ptop_acc[:].rearrange("p (g k) -> p g k", k=TOPK)[:, g0:g1, :],
        )
        nc.sync.dma_start(
            idx_d.ap().rearrange("(g p) k -> p g k", p=P)[:, g0:g1, :],
            idx_acc[:].bitcast(mybir.dt.int32)
            .rearrange("p (g k) -> p g k", k=TOPK)[:, g0:g1, :],
        )
        nc.sync.dma_start(
            routing_d.ap().rearrange("(g p) e -> p g e", p=P)[:, g0:g1, :],
            rout_acc[:].rearrange("p (g e) -> p g e", e=NUM_EXPERTS)[:, g0:g1, :],
        )

    pending_topk = None  # (st, z_sb) of the previous supertile

    for st in range(N_SUP):
        x_tiles = []
        if st == 0:
            # first supertile: load in column pieces so chunk-0 transposes can
            # start after ~1/4 of the bytes; interleave W/b right after the
            # first piece (the first matmul needs W only after 4 transposes)
            for tt in range(TT_PER_SUP):
                xt_in = x_pool.tile([P, HIDDEN], FP32, tag="xin")
                x_tiles.append(xt_in)
            NPIECE = 4
            pw = HIDDEN // NPIECE
            for tt in range(TT_PER_SUP):
                r0 = st * SUP + tt * P
                nc.sync.dma_start(
                    x_tiles[tt][:, 0:pw], x_d.ap()[r0:r0 + P, 0:pw]
                )
            nc.scalar.dma_start(
                w_sb[:].rearrange("p (c e) -> p c e", e=NUM_EXPERTS),
                w_d.ap().rearrange("(c p) e -> p c e", p=P),
            )
            nc.scalar.dma_start(
                b_sb[0:NUM_EXPERTS, :],
                b_d.ap().rearrange("(e one) -> e one", one=1),
            )
            for piece in range(1, NPIECE):
                for tt in range(TT_PER_SUP):
                    r0 = st * SUP + tt * P
                    nc.sync.dma_start(
                        x_tiles[tt][:, piece * pw:(piece + 1) * pw],
                        x_d.ap()[r0:r0 + P, piece * pw:(piece + 1) * pw],
                    )
        else:
            # later supertiles prefetch on the ACT HWDGE ring so the sync
            # ring's sequencer stays free for output flushes
            for tt in range(TT_PER_SUP):
                xt_in = x_pool.tile([P, HIDDEN], FP32, tag="xin")
                r0 = st * SUP + tt * P
                nc.scalar.dma_start(xt_in[:], x_d.ap()[r0:r0 + P, :])
                x_tiles.append(xt_in)

        # split accumulator: even chunks -> rows 0:64 (col group 0), odd
        # chunks -> rows 64:128 (col group 1); the two col-tiled matmuls run
        # concurrently in the PE array (~1.9x measured)
        logits_ps = psl_pool.tile([P, SUP], FP32)

        def emit_mm(c):
            half = c % 2
            nc.tensor.matmul(
                logits_ps[half * NUM_EXPERTS:(half + 1) * NUM_EXPERTS, :],
                w_sb[:, c * NUM_EXPERTS:(c + 1) * NUM_EXPERTS],
                xt_done[c][:],
                start=(c < 2),
                stop=(c >= KC - 2),
                tile_position=(0, half * NUM_EXPERTS),
            )

        # software pipeline by chunk PAIR: transposes+copies for pair p, then
        # the two col-tiled matmuls of pair p-1 back-to-back (adjacent PE
        # instructions are required for the col groups to run concurrently)
        xt_done = {}
        for p in range(KC // 2):
            for c in (2 * p, 2 * p + 1):
                xt_ps = psx_pool.tile([P, SUP], FP32)
                for tt in range(TT_PER_SUP):
                    nc.tensor.transpose(
                        xt_ps[:, tt * P:(tt + 1) * P],
                        x_tiles[tt][:, c * P:(c + 1) * P],
                        ident[:],
                    )
                xt_sb = xt_pool.tile([P, SUP], FP32, tag="xt")
                if c % 2 == 0:
                    nc.scalar.copy(xt_sb[:], xt_ps[:])
                else:
                    nc.vector.tensor_copy(xt_sb[:], xt_ps[:])
                xt_done[c] = xt_sb
            if p >= 1:
                emit_mm(2 * p - 2)
                emit_mm(2 * p - 1)
            if p == 1 and pending_topk is not None:
                emit_topk(*pending_topk)
                pending_topk = None

        emit_mm(KC - 2)
        emit_mm(KC - 1)

        # zz = [L0 + b ; L1], both halves on partitions  [128, 512]
        z_sb = z_pool.tile([P, SUP], FP32, tag="z")
        nc.scalar.activation(
            z_sb[:], logits_ps[:],
            mybir.ActivationFunctionType.Identity, bias=b_sb[:],
        )
        pending_topk = (st, z_sb)

    emit_topk(*pending_topk)


_NC_CACHE = None


def _get_nc():
    global _NC_CACHE
    if _NC_CACHE is None:
        _NC_CACHE = build_nc()
    return _NC_CACHE


def run_sharded(hidden_states, W, b, trace=False):
    nc = _get_nc()
    hs = np.ascontiguousarray(np.asarray(hidden_states, dtype=np.float32))
    W = np.ascontiguousarray(np.asarray(W, dtype=np.float32))
    b = np.ascontiguousarray(np.asarray(b, dtype=np.float32))
    flat = hs.reshape(TOKENS, HIDDEN)
    in_maps = [
        {
            "x": flat[c * TOK_PER_CORE:(c + 1) * TOK_PER_CORE],
            "w": W,
            "b": b,
        }
        for c in range(N_CORES)
    ]
    out = bass_utils.run_bass_kernel_spmd(
        nc, in_maps, core_ids=list(range(N_CORES)), trace=trace
    )
    return out


def kernel(hidden_states, W, b):
    out = run_sharded(hidden_states, W, b)
    res = out.results
    probs = np.concatenate([r["probs"] for r in res], axis=0)
    idx = np.concatenate([r["idx"] for r in res], axis=0)
    routing = np.concatenate([r["routing"] for r in res], axis=0)
    probs_topk = probs.reshape(B, S, TOPK).astype(np.float32)
    indices_topk = idx.reshape(B, S, TOPK).astype(np.int32)
    routing_map = routing.reshape(B, S, NUM_EXPERTS).astype(np.float32)
    return probs_topk, indices_topk, routing_map
